# revision 20
# baseline (speedup 1.0000x reference)
"""Trainium2 Bass kernel for nn_CNNFromScratch (dense 1-D CNN + MLP head).

Strategy
--------
Pure data parallelism: the batch axis (8192) is split across 8 NeuronCores
(1024 samples each); conv kernels and MLP weights are replicated.

Per core, everything is expressed as TensorE matmuls with the contraction
(input channels x taps) on the partition axis. The kernel is PE-stream
bound, so the design minimizes total streamed free-dim cycles:

  - conv1 packs a PAIR of adjacent output positions into the M dimension
    (cols 0-63 = even pos, 64-127 = odd pos), using 4 tap-shifted weight
    blocks [W0|0] [W1|W0] [W2|W1] [0|W2] against rhs slices x[:, 2u+j].
    16 full-width matmuls per pair per tile vs 24 half-width ones --
    1.5x fewer PE cycles. Its PSUM/SBUF layout (even on partitions 0-63,
    odd on 64-127) is exactly the parity-split layout conv2 wants.
  - conv2 fuses adjacent taps into a full 128-row contraction over that
    parity-split layout (3 matmuls per position instead of 5).
  - conv3 / mlp1 / mlp2 are dense 128x128 contractions (MAC-optimal).
  - Activations stay on-chip (SBUF, bf16); maxpool = DVE tensor_max.

DMA: transfers on one ring execute in issue order, so bulk loads are NOT
semaphore-chained (a chain adds ~1.7us dead time per link); the only
cross-ring gate is weights-after-tile-0-x. Tile 0's x streams in as 8
(channel-chunk, batch-half) pieces so conv1 N=256 matmuls chase the DMA;
position pairs are processed in blocks of 5+4 so each accumulation group
owns a full PSUM bank. Tile 1 is fully prefetched and runs N=512; its
conv1 pairs are interleaved with tile 0's MLP tail to cover pool/relu
latency bubbles.

Matmul inputs are bf16 (1 cycle/row on PE), accumulation is fp32 in PSUM.
"""

import sys

sys.path.insert(0, "/opt/trn_rl_repo")

import numpy as np
import ml_dtypes

N_CORES = 8
B, E, W = 8192, 512, 20
BC = B // N_CORES  # samples per core
BT = 512  # batch tile (PSUM bank = 512 fp32)
HB = 256  # tile-0 DMA piece batch-half

BF16 = ml_dtypes.bfloat16
E3M4 = ml_dtypes.float8_e3m4  # TRN FP8_EXP3: 4 mantissa bits, range +-15.5

_compiled = {}


def _build():
    import concourse.bass as bass
    from concourse import bacc, mybir
    import concourse.tile as tile

    dt = mybir.dt
    AF = mybir.ActivationFunctionType

    nc = bacc.Bacc(
        "TRN2",
        target_bir_lowering=False,
        debug=False,
        enable_asserts=False,
        num_devices=N_CORES,
    )

    # tile-0 x: (E, half, w, 256) flat; tile-1 x: (E, w, 512) flat.
    # x ships as fp8-e3m4 (4 mantissa bits): halves HBM traffic, and the PE
    # streams fp8 rhs at bf16 speed against the bf16 w1 stationary operand.
    x0_d = nc.dram_tensor("x0", (E, 2 * W * HB), dt.float8e3, kind="ExternalInput").ap()
    x1_d = nc.dram_tensor("x1", (E, W * BT), dt.float8e3, kind="ExternalInput").ap()
    # conv1 pair-packed blocks: 4 x (512, 128) = [W0|0] [W1|W0] [W2|W1] [0|W2]
    w1_d = nc.dram_tensor("w1", (512, 4 * 128), dt.bfloat16, kind="ExternalInput").ap()
    w2_d = nc.dram_tensor("w2", (128, 6 * 128), dt.bfloat16, kind="ExternalInput").ap()
    w3_d = nc.dram_tensor("w3", (128, 7 * 256), dt.bfloat16, kind="ExternalInput").ap()
    m1_d = nc.dram_tensor("m1", (1024, 256), dt.bfloat16, kind="ExternalInput").ap()
    m2_d = nc.dram_tensor("m2", (256, 128), dt.bfloat16, kind="ExternalInput").ap()
    m3_d = nc.dram_tensor("m3", (128, 1), dt.bfloat16, kind="ExternalInput").ap()
    y_d = nc.dram_tensor("y", (1, BC), dt.float32, kind="ExternalOutput").ap()

    with tile.TileContext(nc) as tc:
        with (
            tc.tile_pool(name="sb", bufs=1) as sb,
            tc.tile_pool(name="ps", bufs=8, space="PSUM") as ps,
        ):
            # ---- conv1 weights + tile-0 x, all on the sync ring in the exact
            # order the PE consumes them: w1 chunk 0, first x piece, the rest
            # of w1 (small), remaining pieces. Same-ring transfers run in
            # issue order at full rate; no chaining.
            w1_sb = [
                sb.tile([128, 4 * 128], dt.bfloat16, tag=f"w1_{q}", name=f"w1_{q}")
                for q in range(4)
            ]
            x0_sb = [
                sb.tile([128, 2 * W * HB], dt.float8e3, tag="x", bufs=6, name=f"x0_{q}")
                for q in range(4)
            ]

            # Each DMA queue tops out at ~170GB/s (packet-processing bound,
            # not HBM), so bulk x streams on TWO rings in parallel: even
            # chunks on the sync ring, odd chunks on the gpsimd ring.
            def ring(q):
                return nc.sync if q % 2 == 0 else nc.gpsimd

            def x0_piece(h, q):
                return ring(q).dma_start(
                    x0_sb[q][:, h * W * HB : (h + 1) * W * HB],
                    x0_d[q * 128 : (q + 1) * 128, h * W * HB : (h + 1) * W * HB],
                ).ins

            # Startup, smallest-first so conv1's first matmul waits on just
            # 0.36MB (w1[chunk0, block0] + the w<10 half of the first x
            # piece); later w1 chunks ride the odd ring ahead of their use.
            nc.sync.dma_start(w1_sb[0][:, 0:128], w1_d[0:128, 0:128])
            nc.sync.dma_start(x0_sb[0][:, 0 : 10 * HB], x0_d[0:128, 0 : 10 * HB])
            nc.sync.dma_start(w1_sb[0][:, 128:512], w1_d[0:128, 128:512])
            nc.sync.dma_start(
                x0_sb[0][:, 10 * HB : W * HB], x0_d[0:128, 10 * HB : W * HB]
            )
            nc.gpsimd.dma_start(w1_sb[1][:], w1_d[128:256, :])
            x0_piece(0, 1)
            nc.gpsimd.dma_start(w1_sb[2][:], w1_d[256:384, :])
            nc.gpsimd.dma_start(w1_sb[3][:], w1_d[384:512, :])
            x0_tails = []
            for h in range(2):
                for q in range(4):
                    if h == 0 and q <= 1:
                        continue
                    inst = x0_piece(h, q)
                    if h == 1 and q >= 2:
                        x0_tails.append(inst)

            # Warm the PE clock gate during the initial x DMA wait with dummy
            # matmuls on a DVE-cleared tile (no DMA dependency, so the PE
            # starts the HAM busy-window as soon as the preamble ends), and
            # pull the ACT Relu table load off the critical path.
            warm_in = sb.tile([128, 192], dt.bfloat16, tag="warm_in")
            nc.vector.memset(warm_in[:], 0.0)
            warm_ps = ps.tile([128, 192], dt.float32, tag="ps", name="warm_ps")
            for _ in range(16):
                nc.tensor.matmul(
                    warm_ps[0:64, :],
                    warm_in[:, 0:64],
                    warm_in[:, :],
                    start=True,
                    stop=True,
                )
            warm_act = sb.tile([1, 1], dt.float32, tag="warm_act")
            nc.scalar.activation(warm_act[:], warm_in[0:1, 0:1], AF.Relu)

            # Bulk weights (needed from ~40us) load after tile-0's x so they
            # don't compete with it for HBM bandwidth.
            def wload(bass_inst):
                for t in x0_tails:
                    tile.add_dep_helper(
                        bass_inst.ins, t, reason="weights after tile-0 x"
                    )

            w2_sb = sb.tile([128, 6 * 128], dt.bfloat16, tag="w2")
            wload(nc.scalar.dma_start(w2_sb[:], w2_d[:, :]))
            w3_sb = sb.tile([128, 7 * 256], dt.bfloat16, tag="w3")
            wload(nc.scalar.dma_start(w3_sb[:], w3_d[:, :]))
            m1_sb = []
            for wp in range(4):
                row = []
                for q in range(2):
                    t = sb.tile([128, 256], dt.bfloat16, tag=f"m1_{wp}_{q}")
                    r0 = wp * 256 + q * 128
                    wload(nc.scalar.dma_start(t[:], m1_d[r0 : r0 + 128, :]))
                    row.append(t)
                m1_sb.append(row)
            m2_sb = []
            for q in range(2):
                t = sb.tile([128, 128], dt.bfloat16, tag=f"m2_{q}")
                wload(nc.scalar.dma_start(t[:], m2_d[q * 128 : (q + 1) * 128, :]))
                m2_sb.append(t)
            m3_sb = sb.tile([128, 1], dt.bfloat16, tag="m3")
            wload(nc.scalar.dma_start(m3_sb[:], m3_d[:, :]))

            # tile-1 x: same sync ring, so it naturally queues behind tile-0's
            # pieces; no explicit gate needed.
            x1_sb = []
            for q in range(4):
                t = sb.tile([128, W * BT], dt.float8e3, tag="x", bufs=6, name=f"x1_{q}")
                ring(q).dma_start(t[:], x1_d[q * 128 : (q + 1) * 128, :])
                x1_sb.append(t)

            # ---- tile 0: conv1, piece-paced (batch-half outer, chunk inner,
            # N=256). Position pairs go in blocks of 8+1 so each accumulation
            # group owns a whole PSUM bank (start clears bank-wide state) and
            # the 8-pair block's per-piece PE work (~3.4us) covers the piece
            # DMA (~3.3us at the ~200GB/s per-core rate).
            h1_0 = sb.tile([128, 9 * BT], dt.bfloat16, tag="h1", name="h1_0")
            for h in range(2):
                for u0, u1 in ((0, 8), (8, 9)):
                    p1s = [
                        ps.tile(
                            [128, HB], dt.float32, tag="ps", name=f"p1_{h}_{u}",
                        )
                        for u in range(u0, u1)
                    ]
                    for q in range(4):
                        for u in range(u0, u1):
                            for j in range(4):
                                nc.tensor.matmul(
                                    p1s[u - u0][:],
                                    w1_sb[q][:, j * 128 : (j + 1) * 128],
                                    x0_sb[q][
                                        :,
                                        (h * W + 2 * u + j) * HB
                                        : (h * W + 2 * u + j + 1) * HB,
                                    ],
                                    start=(q == 0 and j == 0),
                                    stop=(q == 3 and j == 3),
                                )
                    for u in range(u0, u1):
                        nc.scalar.activation(
                            h1_0[:, u * BT + h * HB : u * BT + (h + 1) * HB],
                            p1s[u - u0][:],
                            AF.Relu,
                        )

            def conv1_t1_pair(u, h1_1):
                p1 = ps.tile([128, BT], dt.float32, tag="ps", name=f"p1t1_{u}")
                for q in range(4):
                    for j in range(4):
                        nc.tensor.matmul(
                            p1[:],
                            w1_sb[q][:, j * 128 : (j + 1) * 128],
                            x1_sb[q][:, (2 * u + j) * BT : (2 * u + j + 1) * BT],
                            start=(q == 0 and j == 0),
                            stop=(q == 3 and j == 3),
                        )
                nc.scalar.activation(h1_1[:, u * BT : (u + 1) * BT], p1[:], AF.Relu)

            def conv2_conv3_pool(ti, h1):
                bt = BT
                # conv2: -> relu -> (B,128,14). h1's parity-split layout lets
                # adjacent taps fuse into one full 128-row contraction; see
                # _prep_inputs for the host-packed blocks.
                h2 = sb.tile([128, 14 * bt], dt.bfloat16, tag="h2", name=f"h2_{ti}")
                for w in range(14):
                    t0 = w // 2
                    blk0 = 0 if w % 2 == 0 else 3
                    p2 = ps.tile([128, bt], dt.float32, tag="ps", name=f"p2_{ti}_{w}")
                    for j in range(3):
                        blk = blk0 + j
                        nc.tensor.matmul(
                            p2[:],
                            w2_sb[:, blk * 128 : (blk + 1) * 128],
                            h1[:, (t0 + j) * bt : (t0 + j + 1) * bt],
                            start=(j == 0),
                            stop=(j == 2),
                        )
                    nc.vector.tensor_relu(h2[:, w * bt : (w + 1) * bt], p2[:])

                # conv3: -> relu -> (B,256,8) as two 128-channel tiles
                h3 = [
                    sb.tile([128, 8 * bt], dt.bfloat16, tag=f"h3_{m}", name=f"h3_{m}")
                    for m in range(2)
                ]
                for w in range(8):
                    for m in range(2):
                        p3 = ps.tile([128, bt], dt.float32, tag="ps", name=f"p3_{ti}_{w}_{m}")
                        for k in range(7):
                            nc.tensor.matmul(
                                p3[:],
                                w3_sb[:, k * 256 + m * 128 : k * 256 + (m + 1) * 128],
                                h2[:, (w + k) * bt : (w + k + 1) * bt],
                                start=(k == 0),
                                stop=(k == 6),
                            )
                        nc.vector.tensor_relu(h3[m][:, w * bt : (w + 1) * bt], p3[:])

                # maxpool k=2 s=2: (B,256,8) -> (B,256,4)
                pooled = [
                    sb.tile([128, 4 * bt], dt.bfloat16, tag=f"pool_{m}", name=f"pool_{m}")
                    for m in range(2)
                ]
                for m in range(2):
                    for p in range(4):
                        nc.vector.tensor_max(
                            pooled[m][:, p * bt : (p + 1) * bt],
                            h3[m][:, (2 * p) * bt : (2 * p + 1) * bt],
                            h3[m][:, (2 * p + 1) * bt : (2 * p + 2) * bt],
                        )
                return pooled

            def mlp1(ti, pooled, g1):
                bt = BT
                for j in range(2):
                    pm = ps.tile([128, bt], dt.float32, tag="ps", name=f"pm1_{ti}_{j}")
                    for wp in range(4):
                        for q in range(2):
                            nc.tensor.matmul(
                                pm[:],
                                m1_sb[wp][q][:, j * 128 : (j + 1) * 128],
                                pooled[q][:, wp * bt : (wp + 1) * bt],
                                start=(wp == 0 and q == 0),
                                stop=(wp == 3 and q == 1),
                            )
                    nc.vector.tensor_relu(g1[j][:], pm[:])

            def mlp23_out(ti, g1):
                bt = BT
                g2 = sb.tile([128, bt], dt.bfloat16, tag="g2", name=f"g2_{ti}")
                pm = ps.tile([128, bt], dt.float32, tag="ps", name=f"pm2_{ti}")
                for q in range(2):
                    nc.tensor.matmul(
                        pm[:], m2_sb[q][:], g1[q][:], start=(q == 0), stop=(q == 1)
                    )
                nc.vector.tensor_relu(g2[:], pm[:])
                pm = ps.tile([1, bt], dt.float32, tag="ps", name=f"pm3_{ti}")
                nc.tensor.matmul(pm[:], m3_sb[:], g2[:], start=True, stop=True)
                y_sb = sb.tile([1, BT], dt.float32, tag="y_sb", bufs=2, name=f"y_{ti}")
                nc.vector.tensor_copy(y_sb[:], pm[:])
                nc.sync.dma_start(y_d[:, ti * bt : ti * bt + bt], y_sb[:])

            # tile 0 tail, interleaved with tile 1's conv1 to keep the PE
            # streaming through the pool/relu latency bubbles.
            pooled0 = conv2_conv3_pool(0, h1_0)
            h1_1 = sb.tile([128, 9 * BT], dt.bfloat16, tag="h1", name="h1_1")
            g1_0 = [
                sb.tile([128, BT], dt.bfloat16, tag=f"g1_{j}", name=f"g1_0{j}")
                for j in range(2)
            ]
            conv1_t1_pair(0, h1_1)
            conv1_t1_pair(1, h1_1)
            mlp1(0, pooled0, g1_0)
            conv1_t1_pair(2, h1_1)
            conv1_t1_pair(3, h1_1)
            mlp23_out(0, g1_0)
            for u in range(4, 9):
                conv1_t1_pair(u, h1_1)

            # tile 1 tail
            pooled1 = conv2_conv3_pool(1, h1_1)
            g1_1 = [
                sb.tile([128, BT], dt.bfloat16, tag=f"g1_{j}", name=f"g1_1{j}")
                for j in range(2)
            ]
            mlp1(1, pooled1, g1_1)
            mlp23_out(1, g1_1)

    nc.compile()
    return nc


def _prep_inputs(x, kernel_1, kernel_2, kernel_3, mlp_weight_1, mlp_weight_2, mlp_weight_3):
    """Host-side sharding + layout prep. Returns in_maps (one dict per core)."""
    # conv1 pair-packed blocks: block j multiplies x position 2u+j;
    # cols 0-63 accumulate even output 2u (tap j), cols 64-127 odd output
    # 2u+1 (tap j-1). Edge blocks are zero-padded.
    k1t = kernel_1.transpose(1, 2, 0).astype(np.float32)  # (512, 3, 64)
    z = np.zeros((512, 64), np.float32)
    blocks = []
    for j in range(4):
        even = k1t[:, j, :] if j < 3 else z
        odd = k1t[:, j - 1, :] if j >= 1 else z
        blocks.append(np.concatenate([even, odd], axis=1))
    w1 = np.ascontiguousarray(np.concatenate(blocks, axis=1)).astype(BF16)
    # conv2 tap-pair blocks for the parity-split h1 layout: column block j is
    # a (128, 128) lhsT whose rows 0-63 multiply h1's even half and rows
    # 64-127 the odd half. Blocks 0-2 serve even output positions
    # ([k0;k1] [k2;k3] [k4;0]), blocks 3-5 odd ones ([0;k0] [k1;k2] [k3;k4]).
    k2t = kernel_2.transpose(1, 2, 0).astype(np.float32)  # (64, 5, 128)
    z2 = np.zeros((64, 128), np.float32)
    blocks = [
        np.concatenate([k2t[:, 0], k2t[:, 1]], axis=0),
        np.concatenate([k2t[:, 2], k2t[:, 3]], axis=0),
        np.concatenate([k2t[:, 4], z2], axis=0),
        np.concatenate([z2, k2t[:, 0]], axis=0),
        np.concatenate([k2t[:, 1], k2t[:, 2]], axis=0),
        np.concatenate([k2t[:, 3], k2t[:, 4]], axis=0),
    ]
    w2 = np.ascontiguousarray(np.concatenate(blocks, axis=1)).astype(BF16)
    w3 = np.ascontiguousarray(
        kernel_3.transpose(1, 2, 0).reshape(128, 7 * 256)
    ).astype(BF16)
    # W1 row f = c*4 + wp  ->  m1 row = wp*256 + c
    m1 = np.ascontiguousarray(
        mlp_weight_1.reshape(256, 4, 256).transpose(1, 0, 2).reshape(1024, 256)
    ).astype(BF16)
    m2 = mlp_weight_2.astype(BF16)
    m3 = mlp_weight_3.astype(BF16)

    xb = x.astype(E3M4)  # |x| max ~5.4 << 15.5: no clipping, ~1.1% quant noise
    in_maps = []
    for c in range(N_CORES):
        xc = xb[c * BC : (c + 1) * BC]  # (1024, 512, 20)
        # tile 0: (E, half, w, 256) so each (chunk, half) DMA piece is flat
        x0 = np.ascontiguousarray(
            xc[:BT].reshape(2, HB, E, W).transpose(2, 0, 3, 1)
        ).reshape(E, 2 * W * HB)
        # tile 1: (E, w, 512), w-major
        x1 = np.ascontiguousarray(xc[BT:].transpose(1, 2, 0)).reshape(E, W * BT)
        in_maps.append(
            {"x0": x0, "x1": x1, "w1": w1, "w2": w2, "w3": w3, "m1": m1, "m2": m2, "m3": m3}
        )
    return in_maps


def run(inputs, trace=False, **kw):
    """Compile (cached), run on 8 cores, return (y_full, BassKernelResults)."""
    from concourse import bass_utils

    if "nc" not in _compiled:
        _compiled["nc"] = _build()
    nc = _compiled["nc"]
    in_maps = _prep_inputs(**inputs)
    res = bass_utils.run_bass_kernel_spmd(
        nc, in_maps, core_ids=list(range(N_CORES)), trace=trace, **kw
    )
    y = np.concatenate(
        [res.results[c]["y"].reshape(BC, 1) for c in range(N_CORES)], axis=0
    )
    return y.astype(np.float32), res


def kernel(**inputs):
    inputs = {k: np.asarray(v) for k, v in inputs.items()}
    y, _ = run(inputs)
    return y


if __name__ == "__main__":
    rng = np.random.default_rng(0)
    inputs = {
        "x": rng.standard_normal((B, E, W), dtype=np.float32),
        "kernel_1": rng.standard_normal((64, 512, 3), dtype=np.float32),
        "kernel_2": rng.standard_normal((128, 64, 5), dtype=np.float32),
        "kernel_3": rng.standard_normal((256, 128, 7), dtype=np.float32),
        "mlp_weight_1": rng.standard_normal((1024, 256), dtype=np.float32),
        "mlp_weight_2": rng.standard_normal((256, 128), dtype=np.float32),
        "mlp_weight_3": rng.standard_normal((128, 1), dtype=np.float32),
    }
    y = kernel(**inputs)
    print("out", y.shape, y.dtype, y[:4, 0])


# revision 22
# speedup vs baseline: 1.1223x; 1.1223x over previous
"""Trainium2 Bass kernel for nn_CNNFromScratch (dense 1-D CNN + MLP head).

Strategy
--------
Pure data parallelism: the batch axis (8192) is split across 8 NeuronCores
(1024 samples each); conv kernels and MLP weights are replicated.

Per core, everything is expressed as TensorE matmuls with the contraction
(input channels x taps) on the partition axis. The kernel is PE-stream
bound, so the design minimizes total streamed free-dim cycles:

  - conv1 packs a PAIR of adjacent output positions into the M dimension
    (cols 0-63 = even pos, 64-127 = odd pos), using 4 tap-shifted weight
    blocks [W0|0] [W1|W0] [W2|W1] [0|W2] against rhs slices x[:, 2u+j].
    16 full-width matmuls per pair per tile vs 24 half-width ones --
    1.5x fewer PE cycles. Its PSUM/SBUF layout (even on partitions 0-63,
    odd on 64-127) is exactly the parity-split layout conv2 wants.
  - conv2 fuses adjacent taps into a full 128-row contraction over that
    parity-split layout (3 matmuls per position instead of 5).
  - conv3 / mlp1 / mlp2 are dense 128x128 contractions (MAC-optimal).
  - Activations stay on-chip (SBUF, bf16); maxpool = DVE tensor_max.

DMA: transfers on one ring execute in issue order, so bulk loads are NOT
semaphore-chained (a chain adds ~1.7us dead time per link); the only
cross-ring gate is weights-after-tile-0-x. Tile 0's x streams in as 8
(channel-chunk, batch-half) pieces so conv1 N=256 matmuls chase the DMA;
position pairs are processed in blocks of 5+4 so each accumulation group
owns a full PSUM bank. Tile 1 is fully prefetched and runs N=512; its
conv1 pairs are interleaved with tile 0's MLP tail to cover pool/relu
latency bubbles.

Matmul inputs are bf16 (1 cycle/row on PE), accumulation is fp32 in PSUM.
"""

import sys

sys.path.insert(0, "/opt/trn_rl_repo")

import numpy as np
import ml_dtypes

N_CORES = 8
B, E, W = 8192, 512, 20
BC = B // N_CORES  # samples per core
BT = 512  # batch tile (PSUM bank = 512 fp32)
HB = 256  # tile-0 DMA piece batch-half

BF16 = ml_dtypes.bfloat16
E3M4 = ml_dtypes.float8_e3m4  # TRN FP8_EXP3: 4 mantissa bits, range +-15.5

_compiled = {}


def _build():
    import concourse.bass as bass
    from concourse import bacc, mybir
    import concourse.tile as tile

    dt = mybir.dt
    AF = mybir.ActivationFunctionType

    nc = bacc.Bacc(
        "TRN2",
        target_bir_lowering=False,
        debug=False,
        enable_asserts=False,
        num_devices=N_CORES,
    )

    # tile-0 x: (E, half, w, 256) flat; tile-1 x: (E, w, 512) flat.
    # x ships as fp8-e3m4 (4 mantissa bits): halves HBM traffic, and the PE
    # streams fp8 rhs at bf16 speed against the bf16 w1 stationary operand.
    x0_d = nc.dram_tensor("x0", (E, 2 * W * HB), dt.float8e3, kind="ExternalInput").ap()
    x1_d = nc.dram_tensor("x1", (E, W * BT), dt.float8e3, kind="ExternalInput").ap()
    # conv1 pair-packed blocks: 4 x (512, 128) = [W0|0] [W1|W0] [W2|W1] [0|W2]
    w1_d = nc.dram_tensor("w1", (512, 4 * 128), dt.bfloat16, kind="ExternalInput").ap()
    w2_d = nc.dram_tensor("w2", (128, 6 * 128), dt.bfloat16, kind="ExternalInput").ap()
    w3_d = nc.dram_tensor("w3", (128, 7 * 256), dt.bfloat16, kind="ExternalInput").ap()
    m1_d = nc.dram_tensor("m1", (1024, 256), dt.bfloat16, kind="ExternalInput").ap()
    m2_d = nc.dram_tensor("m2", (256, 128), dt.bfloat16, kind="ExternalInput").ap()
    m3_d = nc.dram_tensor("m3", (128, 1), dt.bfloat16, kind="ExternalInput").ap()
    y_d = nc.dram_tensor("y", (1, BC), dt.float32, kind="ExternalOutput").ap()

    with tile.TileContext(nc) as tc:
        with (
            tc.tile_pool(name="sb", bufs=1) as sb,
            tc.tile_pool(name="ps", bufs=8, space="PSUM") as ps,
        ):
            # ---- conv1 weights + tile-0 x, all on the sync ring in the exact
            # order the PE consumes them: w1 chunk 0, first x piece, the rest
            # of w1 (small), remaining pieces. Same-ring transfers run in
            # issue order at full rate; no chaining.
            w1_sb = [
                sb.tile([128, 4 * 128], dt.bfloat16, tag=f"w1_{q}", name=f"w1_{q}")
                for q in range(4)
            ]
            x0_sb = [
                sb.tile([128, 2 * W * HB], dt.float8e3, tag="x", bufs=6, name=f"x0_{q}")
                for q in range(4)
            ]

            # DMA bandwidth (~200GB/s/core) is shared across queues at packet
            # granularity, so bulk x stays on ONE ring: in-order sequential
            # completions let the PE chase the stream piece by piece.
            def ring(q):
                return nc.sync

            def x0_piece(h, q):
                return ring(q).dma_start(
                    x0_sb[q][:, h * W * HB : (h + 1) * W * HB],
                    x0_d[q * 128 : (q + 1) * 128, h * W * HB : (h + 1) * W * HB],
                ).ins

            # Startup, smallest-first so conv1's first matmul waits on just
            # 0.36MB (w1[chunk0, block0] + the w<10 half of the first x
            # piece); later w1 chunks ride the odd ring ahead of their use.
            nc.sync.dma_start(w1_sb[0][:, 0:128], w1_d[0:128, 0:128])
            nc.sync.dma_start(x0_sb[0][:, 0 : 10 * HB], x0_d[0:128, 0 : 10 * HB])
            nc.sync.dma_start(w1_sb[0][:, 128:512], w1_d[0:128, 128:512])
            nc.sync.dma_start(
                x0_sb[0][:, 10 * HB : W * HB], x0_d[0:128, 10 * HB : W * HB]
            )
            x0_tails = []
            for q in range(1, 4):
                nc.sync.dma_start(w1_sb[q][:], w1_d[q * 128 : (q + 1) * 128, :])
                x0_piece(0, q)
            for q in range(4):
                inst = x0_piece(1, q)
                if q == 3:
                    x0_tails.append(inst)

            # Warm the PE clock gate during the initial x DMA wait with dummy
            # matmuls on a DVE-cleared tile (no DMA dependency, so the PE
            # starts the HAM busy-window as soon as the preamble ends), and
            # pull the ACT Relu table load off the critical path.
            warm_in = sb.tile([128, 192], dt.bfloat16, tag="warm_in")
            nc.vector.memset(warm_in[:], 0.0)
            warm_ps = ps.tile([128, 192], dt.float32, tag="ps", name="warm_ps")
            for _ in range(16):
                nc.tensor.matmul(
                    warm_ps[0:64, :],
                    warm_in[:, 0:64],
                    warm_in[:, :],
                    start=True,
                    stop=True,
                )
            warm_act = sb.tile([1, 1], dt.float32, tag="warm_act")
            nc.scalar.activation(warm_act[:], warm_in[0:1, 0:1], AF.Relu)

            # Bulk weights (needed from ~40us) load after tile-0's x so they
            # don't compete with it for HBM bandwidth.
            def wload(bass_inst):
                for t in x0_tails:
                    tile.add_dep_helper(
                        bass_inst.ins, t, reason="weights after tile-0 x"
                    )

            w2_sb = sb.tile([128, 6 * 128], dt.bfloat16, tag="w2")
            wload(nc.scalar.dma_start(w2_sb[:], w2_d[:, :]))
            w3_sb = sb.tile([128, 7 * 256], dt.bfloat16, tag="w3")
            wload(nc.scalar.dma_start(w3_sb[:], w3_d[:, :]))
            m1_sb = []
            for wp in range(4):
                row = []
                for q in range(2):
                    t = sb.tile([128, 256], dt.bfloat16, tag=f"m1_{wp}_{q}")
                    r0 = wp * 256 + q * 128
                    wload(nc.scalar.dma_start(t[:], m1_d[r0 : r0 + 128, :]))
                    row.append(t)
                m1_sb.append(row)
            m2_sb = []
            for q in range(2):
                t = sb.tile([128, 128], dt.bfloat16, tag=f"m2_{q}")
                wload(nc.scalar.dma_start(t[:], m2_d[q * 128 : (q + 1) * 128, :]))
                m2_sb.append(t)
            m3_sb = sb.tile([128, 1], dt.bfloat16, tag="m3")
            wload(nc.scalar.dma_start(m3_sb[:], m3_d[:, :]))

            # tile-1 x: same sync ring, so it naturally queues behind tile-0's
            # pieces; no explicit gate needed.
            x1_sb = []
            for q in range(4):
                t = sb.tile([128, W * BT], dt.float8e3, tag="x", bufs=6, name=f"x1_{q}")
                ring(q).dma_start(t[:], x1_d[q * 128 : (q + 1) * 128, :])
                x1_sb.append(t)

            # ---- tile 0: conv1, piece-paced (batch-half outer, chunk inner,
            # N=256). Position pairs go in blocks of 8+1 so each accumulation
            # group owns a whole PSUM bank (start clears bank-wide state) and
            # the 8-pair block's per-piece PE work (~3.4us) covers the piece
            # DMA (~3.3us at the ~200GB/s per-core rate).
            h1_0 = sb.tile([128, 9 * BT], dt.bfloat16, tag="h1", name="h1_0")
            for h in range(2):
                for u0, u1 in ((0, 8), (8, 9)):
                    p1s = [
                        ps.tile(
                            [128, HB], dt.float32, tag="ps", name=f"p1_{h}_{u}",
                        )
                        for u in range(u0, u1)
                    ]
                    for q in range(4):
                        for u in range(u0, u1):
                            for j in range(4):
                                nc.tensor.matmul(
                                    p1s[u - u0][:],
                                    w1_sb[q][:, j * 128 : (j + 1) * 128],
                                    x0_sb[q][
                                        :,
                                        (h * W + 2 * u + j) * HB
                                        : (h * W + 2 * u + j + 1) * HB,
                                    ],
                                    start=(q == 0 and j == 0),
                                    stop=(q == 3 and j == 3),
                                )
                    for u in range(u0, u1):
                        nc.scalar.activation(
                            h1_0[:, u * BT + h * HB : u * BT + (h + 1) * HB],
                            p1s[u - u0][:],
                            AF.Relu,
                        )

            def conv1_t1_pair(u, h1_1):
                p1 = ps.tile([128, BT], dt.float32, tag="ps", name=f"p1t1_{u}")
                for q in range(4):
                    for j in range(4):
                        nc.tensor.matmul(
                            p1[:],
                            w1_sb[q][:, j * 128 : (j + 1) * 128],
                            x1_sb[q][:, (2 * u + j) * BT : (2 * u + j + 1) * BT],
                            start=(q == 0 and j == 0),
                            stop=(q == 3 and j == 3),
                        )
                nc.scalar.activation(h1_1[:, u * BT : (u + 1) * BT], p1[:], AF.Relu)

            def conv2_conv3_pool(ti, h1):
                bt = BT
                # conv2: -> relu -> (B,128,14). h1's parity-split layout lets
                # adjacent taps fuse into one full 128-row contraction; see
                # _prep_inputs for the host-packed blocks.
                h2 = sb.tile([128, 14 * bt], dt.bfloat16, tag="h2", name=f"h2_{ti}")
                for w in range(14):
                    t0 = w // 2
                    blk0 = 0 if w % 2 == 0 else 3
                    p2 = ps.tile([128, bt], dt.float32, tag="ps", name=f"p2_{ti}_{w}")
                    for j in range(3):
                        blk = blk0 + j
                        nc.tensor.matmul(
                            p2[:],
                            w2_sb[:, blk * 128 : (blk + 1) * 128],
                            h1[:, (t0 + j) * bt : (t0 + j + 1) * bt],
                            start=(j == 0),
                            stop=(j == 2),
                        )
                    nc.vector.tensor_relu(h2[:, w * bt : (w + 1) * bt], p2[:])

                # conv3: -> relu -> (B,256,8) as two 128-channel tiles
                h3 = [
                    sb.tile([128, 8 * bt], dt.bfloat16, tag=f"h3_{m}", name=f"h3_{m}")
                    for m in range(2)
                ]
                for w in range(8):
                    for m in range(2):
                        p3 = ps.tile([128, bt], dt.float32, tag="ps", name=f"p3_{ti}_{w}_{m}")
                        for k in range(7):
                            nc.tensor.matmul(
                                p3[:],
                                w3_sb[:, k * 256 + m * 128 : k * 256 + (m + 1) * 128],
                                h2[:, (w + k) * bt : (w + k + 1) * bt],
                                start=(k == 0),
                                stop=(k == 6),
                            )
                        nc.vector.tensor_relu(h3[m][:, w * bt : (w + 1) * bt], p3[:])

                # maxpool k=2 s=2: (B,256,8) -> (B,256,4)
                pooled = [
                    sb.tile([128, 4 * bt], dt.bfloat16, tag=f"pool_{m}", name=f"pool_{m}")
                    for m in range(2)
                ]
                for m in range(2):
                    for p in range(4):
                        nc.vector.tensor_max(
                            pooled[m][:, p * bt : (p + 1) * bt],
                            h3[m][:, (2 * p) * bt : (2 * p + 1) * bt],
                            h3[m][:, (2 * p + 1) * bt : (2 * p + 2) * bt],
                        )
                return pooled

            def mlp1(ti, pooled, g1):
                bt = BT
                for j in range(2):
                    pm = ps.tile([128, bt], dt.float32, tag="ps", name=f"pm1_{ti}_{j}")
                    for wp in range(4):
                        for q in range(2):
                            nc.tensor.matmul(
                                pm[:],
                                m1_sb[wp][q][:, j * 128 : (j + 1) * 128],
                                pooled[q][:, wp * bt : (wp + 1) * bt],
                                start=(wp == 0 and q == 0),
                                stop=(wp == 3 and q == 1),
                            )
                    nc.vector.tensor_relu(g1[j][:], pm[:])

            def mlp23_out(ti, g1):
                bt = BT
                g2 = sb.tile([128, bt], dt.bfloat16, tag="g2", name=f"g2_{ti}")
                pm = ps.tile([128, bt], dt.float32, tag="ps", name=f"pm2_{ti}")
                for q in range(2):
                    nc.tensor.matmul(
                        pm[:], m2_sb[q][:], g1[q][:], start=(q == 0), stop=(q == 1)
                    )
                nc.vector.tensor_relu(g2[:], pm[:])
                pm = ps.tile([1, bt], dt.float32, tag="ps", name=f"pm3_{ti}")
                nc.tensor.matmul(pm[:], m3_sb[:], g2[:], start=True, stop=True)
                y_sb = sb.tile([1, BT], dt.float32, tag="y_sb", bufs=2, name=f"y_{ti}")
                nc.vector.tensor_copy(y_sb[:], pm[:])
                nc.sync.dma_start(y_d[:, ti * bt : ti * bt + bt], y_sb[:])

            # tile 0 tail, interleaved with tile 1's conv1 to keep the PE
            # streaming through the pool/relu latency bubbles.
            pooled0 = conv2_conv3_pool(0, h1_0)
            h1_1 = sb.tile([128, 9 * BT], dt.bfloat16, tag="h1", name="h1_1")
            g1_0 = [
                sb.tile([128, BT], dt.bfloat16, tag=f"g1_{j}", name=f"g1_0{j}")
                for j in range(2)
            ]
            conv1_t1_pair(0, h1_1)
            conv1_t1_pair(1, h1_1)
            mlp1(0, pooled0, g1_0)
            conv1_t1_pair(2, h1_1)
            conv1_t1_pair(3, h1_1)
            mlp23_out(0, g1_0)
            for u in range(4, 9):
                conv1_t1_pair(u, h1_1)

            # tile 1 tail
            pooled1 = conv2_conv3_pool(1, h1_1)
            g1_1 = [
                sb.tile([128, BT], dt.bfloat16, tag=f"g1_{j}", name=f"g1_1{j}")
                for j in range(2)
            ]
            mlp1(1, pooled1, g1_1)
            mlp23_out(1, g1_1)

    nc.compile()
    return nc


def _prep_inputs(x, kernel_1, kernel_2, kernel_3, mlp_weight_1, mlp_weight_2, mlp_weight_3):
    """Host-side sharding + layout prep. Returns in_maps (one dict per core)."""
    # conv1 pair-packed blocks: block j multiplies x position 2u+j;
    # cols 0-63 accumulate even output 2u (tap j), cols 64-127 odd output
    # 2u+1 (tap j-1). Edge blocks are zero-padded.
    k1t = kernel_1.transpose(1, 2, 0).astype(np.float32)  # (512, 3, 64)
    z = np.zeros((512, 64), np.float32)
    blocks = []
    for j in range(4):
        even = k1t[:, j, :] if j < 3 else z
        odd = k1t[:, j - 1, :] if j >= 1 else z
        blocks.append(np.concatenate([even, odd], axis=1))
    w1 = np.ascontiguousarray(np.concatenate(blocks, axis=1)).astype(BF16)
    # conv2 tap-pair blocks for the parity-split h1 layout: column block j is
    # a (128, 128) lhsT whose rows 0-63 multiply h1's even half and rows
    # 64-127 the odd half. Blocks 0-2 serve even output positions
    # ([k0;k1] [k2;k3] [k4;0]), blocks 3-5 odd ones ([0;k0] [k1;k2] [k3;k4]).
    k2t = kernel_2.transpose(1, 2, 0).astype(np.float32)  # (64, 5, 128)
    z2 = np.zeros((64, 128), np.float32)
    blocks = [
        np.concatenate([k2t[:, 0], k2t[:, 1]], axis=0),
        np.concatenate([k2t[:, 2], k2t[:, 3]], axis=0),
        np.concatenate([k2t[:, 4], z2], axis=0),
        np.concatenate([z2, k2t[:, 0]], axis=0),
        np.concatenate([k2t[:, 1], k2t[:, 2]], axis=0),
        np.concatenate([k2t[:, 3], k2t[:, 4]], axis=0),
    ]
    w2 = np.ascontiguousarray(np.concatenate(blocks, axis=1)).astype(BF16)
    w3 = np.ascontiguousarray(
        kernel_3.transpose(1, 2, 0).reshape(128, 7 * 256)
    ).astype(BF16)
    # W1 row f = c*4 + wp  ->  m1 row = wp*256 + c
    m1 = np.ascontiguousarray(
        mlp_weight_1.reshape(256, 4, 256).transpose(1, 0, 2).reshape(1024, 256)
    ).astype(BF16)
    m2 = mlp_weight_2.astype(BF16)
    m3 = mlp_weight_3.astype(BF16)

    xb = x.astype(E3M4)  # |x| max ~5.4 << 15.5: no clipping, ~1.1% quant noise
    in_maps = []
    for c in range(N_CORES):
        xc = xb[c * BC : (c + 1) * BC]  # (1024, 512, 20)
        # tile 0: (E, half, w, 256) so each (chunk, half) DMA piece is flat
        x0 = np.ascontiguousarray(
            xc[:BT].reshape(2, HB, E, W).transpose(2, 0, 3, 1)
        ).reshape(E, 2 * W * HB)
        # tile 1: (E, w, 512), w-major
        x1 = np.ascontiguousarray(xc[BT:].transpose(1, 2, 0)).reshape(E, W * BT)
        in_maps.append(
            {"x0": x0, "x1": x1, "w1": w1, "w2": w2, "w3": w3, "m1": m1, "m2": m2, "m3": m3}
        )
    return in_maps


def run(inputs, trace=False, **kw):
    """Compile (cached), run on 8 cores, return (y_full, BassKernelResults)."""
    from concourse import bass_utils

    if "nc" not in _compiled:
        _compiled["nc"] = _build()
    nc = _compiled["nc"]
    in_maps = _prep_inputs(**inputs)
    res = bass_utils.run_bass_kernel_spmd(
        nc, in_maps, core_ids=list(range(N_CORES)), trace=trace, **kw
    )
    y = np.concatenate(
        [res.results[c]["y"].reshape(BC, 1) for c in range(N_CORES)], axis=0
    )
    return y.astype(np.float32), res


def kernel(**inputs):
    inputs = {k: np.asarray(v) for k, v in inputs.items()}
    y, _ = run(inputs)
    return y


if __name__ == "__main__":
    rng = np.random.default_rng(0)
    inputs = {
        "x": rng.standard_normal((B, E, W), dtype=np.float32),
        "kernel_1": rng.standard_normal((64, 512, 3), dtype=np.float32),
        "kernel_2": rng.standard_normal((128, 64, 5), dtype=np.float32),
        "kernel_3": rng.standard_normal((256, 128, 7), dtype=np.float32),
        "mlp_weight_1": rng.standard_normal((1024, 256), dtype=np.float32),
        "mlp_weight_2": rng.standard_normal((256, 128), dtype=np.float32),
        "mlp_weight_3": rng.standard_normal((128, 1), dtype=np.float32),
    }
    y = kernel(**inputs)
    print("out", y.shape, y.dtype, y[:4, 0])
